# revision 1
# baseline (speedup 1.0000x reference)
"""Dual-score causal attention on 8 Trainium2 NeuronCores.

Math (per batch*head):
    S = (q @ k.T + pe_q @ pe_k.T) * D**-0.5   == concat(q,pe_q) @ concat(k,pe_k).T * scale
    O = softmax(causal_mask(S)) @ v

Sharding: B*H = 32 pairs -> 4 per core (head/data parallel, no collectives).

Per-core kernel layout choices:
  - Q' = [q|pe_q], K' = [k|pe_k] have head dim 128 = PE contraction width.
  - Compute S^T tiles [128 k x 512 q] so that both the softmax denominator and
    the A@V contraction run over the partition axis (ones-column trick: V' =
    [V|1] gives row sums from the same matmul chain, no vector reductions).
  - fp16 operands (full PE rate; max rel err ~3e-4 vs fp32 reference), fp32
    accumulation in PSUM.  exp() needs no max-subtraction: scores are ~N(0,2)
    and bounded by ~8 so exp is within fp16/fp32 range.
  - Q/K reach d-major [128 d', L] SBUF layout via SWDGE cast-DMA (f32->f16)
    into a natural-layout staging tile + xbar DMA-transpose of [128,128] tiles.
  - Causality: fully-masked k-blocks are skipped; partially-masked (diagonal)
    tiles trim the dead query columns in the matmul and fix the 128x128
    triangle with a 0/1 fp16 multiply on VectorE.
  - O^T [65, 512] (row 64 = softmax denominator) is transposed back on PE via
    identity matmul, then normalized with a per-partition reciprocal multiply.
"""

import os
import sys

import numpy as np

B, H, L, D = 2, 16, 2048, 64
NCORES = 8
BHPC = (B * H) // NCORES  # bh pairs per core = 4
QB = 512  # query block (S^T free dim)
KB = 128  # key block (S^T partition dim)
NQB = L // QB  # 4
NKB = L // KB  # 16
KB_PER_QB = QB // KB  # 4
SCALE = float(D) ** -0.5

_CACHE = {}


def _import_concourse():
    try:
        import concourse  # noqa: F401
    except ImportError:
        for p in ("/opt/trn_rl_repo", "/root/.axon_site/_ro/trn_rl_repo"):
            if os.path.isdir(p) and p not in sys.path:
                sys.path.insert(0, p)


def _build_nc():
    """Build the single-core Bass program (same NEFF for all 8 cores)."""
    _import_concourse()
    from contextlib import ExitStack

    import concourse.tile as tile
    from concourse import bacc, mybir

    f32 = mybir.dt.float32
    f16 = mybir.dt.float16

    # Bacc (not raw Bass): its compile() legalizes the 1-wait-per-instruction
    # TRN2 constraint by splitting waits onto nop/event instructions
    nc = bacc.Bacc("TRN2", target_bir_lowering=False, debug=False)

    # qpe/kpe are host-side concat([q, pe_q], -1): one producer DMA per stage
    # tile keeps the xbar-transpose instructions (very few ISA sync-wait
    # slots) at <=1 wait each
    qpe_d = nc.dram_tensor("qpe", [BHPC, L, 2 * D], f32, kind="ExternalInput").ap()
    kpe_d = nc.dram_tensor("kpe", [BHPC, L, 2 * D], f32, kind="ExternalInput").ap()
    v_d = nc.dram_tensor("v", [BHPC, L, D], f32, kind="ExternalInput").ap()
    tri_d = nc.dram_tensor("tri", [128, 128], f16, kind="ExternalInput").ap()
    ident_d = nc.dram_tensor("ident", [128, 128], f32, kind="ExternalInput").ap()
    out_d = nc.dram_tensor("out", [BHPC, L, D], f32, kind="ExternalOutput").ap()

    Exp = mybir.ActivationFunctionType.Exp

    with tile.TileContext(nc) as tc:
        with ExitStack() as ctx:
            ep = ctx.enter_context

            const_pool = ep(tc.tile_pool(name="const", bufs=1))
            dstage_pool = ep(tc.tile_pool(name="dstage", bufs=4, space="DRAM"))
            qT_pool = ep(tc.tile_pool(name="qT", bufs=BHPC))
            kT_pool = ep(tc.tile_pool(name="kT", bufs=BHPC))
            v_pool = ep(tc.tile_pool(name="v", bufs=2))
            ex_pool = ep(tc.tile_pool(name="ex", bufs=6))
            otsb_pool = ep(tc.tile_pool(name="otsb", bufs=2))
            ost_pool = ep(tc.tile_pool(name="ost", bufs=2))
            rc_pool = ep(tc.tile_pool(name="rc", bufs=4))
            stp_pool = ep(tc.tile_pool(name="stp", bufs=2, space="PSUM"))
            otp_pool = ep(tc.tile_pool(name="otp", bufs=2, space="PSUM"))
            tp_pool = ep(tc.tile_pool(name="tp", bufs=2, space="PSUM"))

            tri = const_pool.tile([128, 128], f16)
            nc.gpsimd.dma_start(tri[:], tri_d)
            ident = const_pool.tile([128, 128], f32)
            nc.gpsimd.dma_start(ident[:], ident_d)

            for bh in range(BHPC):
                # ---- load + transpose Q', K' to d-major [128, L] ----
                # cast-DMA f32->f16 into a contiguous DRAM staging copy, then
                # one big DRAM->SBUF xbar transpose per tensor (the xbar only
                # streams near line-rate from contiguous DRAM sources; small
                # SBUF-source tiles serialize at ~1.2us each)
                qT = qT_pool.tile([128, L], f16)
                kT = kT_pool.tile([128, L], f16)
                if bh == 0:
                    # startup fast-path: cast+transpose in halves so the
                    # first S matmuls (which need only the first 1024
                    # columns) start ~15us earlier; later bh keep single
                    # big transposes to minimize mid-kernel hazard-guard
                    # serialization points
                    dq = dstage_pool.tile([L, 2 * D], f16, tag="dst")
                    dk = dstage_pool.tile([L, 2 * D], f16, tag="dst")
                    half = L // 2
                    for h0 in (0, half):
                        sl = slice(h0, h0 + half)
                        nc.gpsimd.dma_start(dk[sl, :], kpe_d[bh, sl, :])
                        nc.gpsimd.dma_start(dq[sl, :], qpe_d[bh, sl, :])
                        nc.sync.dma_start_transpose(kT[:, sl], dk[sl, :])
                        nc.sync.dma_start_transpose(qT[:, sl], dq[sl, :])
                else:
                    for tT, src in ((qT, qpe_d), (kT, kpe_d)):
                        dst = dstage_pool.tile([L, 2 * D], f16, tag="dst")
                        nc.gpsimd.dma_start(dst[:], src[bh])
                        nc.sync.dma_start_transpose(tT[:], dst[:])
                vsb = v_pool.tile([128, NKB, D + 1], f16)
                nc.vector.memset(vsb[:, :, D : D + 1], 1.0)
                nc.gpsimd.dma_start(
                    vsb[:, :, 0:D],
                    v_d[bh].rearrange("(n p) d -> p n d", p=128),
                )

                ost = ost_pool.tile([128, NKB, D], f32)
                for qi in range(NQB):
                    otp = otp_pool.tile([D + 1, QB], f32)
                    nfull = KB_PER_QB * qi  # fully-unmasked k-blocks

                    # stage list: full tiles in pairs (one [128,1024] exp per
                    # pair halves the per-ACTIVATE overhead), then the four
                    # partially-masked diagonal tiles singly
                    stages = [("pair", j0) for j0 in range(0, nfull, 2)]
                    stages += [("dpair", r0) for r0 in range(0, KB_PER_QB, 2)]

                    def emit_s(stage):
                        kind, a = stage
                        stp = stp_pool.tile([128, 2 * QB], f32, tag="stp")
                        ex = ex_pool.tile([128, 2 * QB], f16, tag="ex")
                        if kind == "pair":
                            for h_ in (0, 1):
                                j = a + h_
                                nc.tensor.matmul(
                                    stp[:, h_ * QB : (h_ + 1) * QB],
                                    lhsT=kT[:, j * KB : (j + 1) * KB],
                                    rhs=qT[:, qi * QB : (qi + 1) * QB],
                                    start=True,
                                    stop=True,
                                    skip_group_check=True,
                                )
                            nc.scalar.activation(ex[:], stp[:], Exp, scale=SCALE)
                        else:
                            # two diagonal blocks r0, r0+1 packed into one
                            # activation: [0:na) for r0, [na:na+nb) for r0+1
                            off = 0
                            for r_ in (a, a + 1):
                                j = nfull + r_
                                m = KB * r_
                                n = QB - m
                                nc.tensor.matmul(
                                    stp[:, off : off + n],
                                    lhsT=kT[:, j * KB : (j + 1) * KB],
                                    rhs=qT[:, qi * QB + m : (qi + 1) * QB],
                                    start=True,
                                    stop=True,
                                    skip_group_check=True,
                                )
                                off += n
                            nc.scalar.activation(
                                ex[:, 0:off], stp[:, 0:off], Exp, scale=SCALE
                            )
                            # triangle fix on each block's leading 128 cols
                            na = QB - KB * a
                            nc.vector.tensor_mul(ex[:, 0:KB], ex[:, 0:KB], tri[:])
                            nc.vector.tensor_mul(
                                ex[:, na : na + KB], ex[:, na : na + KB], tri[:]
                            )
                        return ex

                    def emit_av(stage, ex, first, last):
                        kind, a = stage
                        if kind == "pair":
                            for h_ in (0, 1):
                                j = a + h_
                                nc.tensor.matmul(
                                    otp[:],
                                    lhsT=vsb[:, j, :],
                                    rhs=ex[:, h_ * QB : (h_ + 1) * QB],
                                    start=first and h_ == 0,
                                    stop=last and h_ == 1,
                                    skip_group_check=True,
                                )
                        else:
                            off = 0
                            for r_ in (a, a + 1):
                                j = nfull + r_
                                m = KB * r_
                                n = QB - m
                                nc.tensor.matmul(
                                    otp[:, m:QB],
                                    lhsT=vsb[:, j, :],
                                    rhs=ex[:, off : off + n],
                                    start=first and r_ == a,
                                    stop=last and r_ == a + 1,
                                    skip_group_check=True,
                                )
                                off += n

                    # software pipeline: keep PE fed with S-matmuls while the
                    # scalar engine computes exp of earlier tiles
                    LAG = 2
                    nst = len(stages)
                    exs = {}
                    for t in range(nst + LAG):
                        if t < nst:
                            exs[t] = emit_s(stages[t])
                        if t >= LAG:
                            s_ = t - LAG
                            emit_av(
                                stages[s_], exs.pop(s_),
                                first=(s_ == 0), last=(s_ == nst - 1),
                            )
                    otsb = otsb_pool.tile([D + 1, QB], f32)
                    nc.vector.tensor_copy(otsb[:], otp[:])
                    for c in range(KB_PER_QB):
                        op = tp_pool.tile([128, D + 1], f32, tag="tp")
                        nc.tensor.transpose(
                            op[:],
                            otsb[:, c * 128 : (c + 1) * 128],
                            ident[0 : D + 1, 0 : D + 1],
                        )
                        rc = rc_pool.tile([128, 1], f32)
                        nc.vector.reciprocal(rc[:], op[:, D : D + 1])
                        nc.vector.tensor_scalar_mul(
                            ost[:, qi * KB_PER_QB + c, :], op[:, 0:D], rc[:]
                        )
                nc.gpsimd.dma_start(
                    out_d[bh].rearrange("(n p) d -> p n d", p=128), ost[:]
                )

    nc.compile()
    return nc


def _host_consts():
    kk = np.arange(128)[:, None]
    cc = np.arange(128)[None, :]
    tri = (kk <= cc).astype(np.float16)
    ident = np.eye(128, dtype=np.float32)
    return tri, ident


def _shard_inputs(q, k, v, pe_q, pe_k):
    q = np.asarray(q, dtype=np.float32).reshape(B * H, L, D)
    k = np.asarray(k, dtype=np.float32).reshape(B * H, L, D)
    v = np.ascontiguousarray(np.asarray(v, dtype=np.float32)).reshape(B * H, L, D)
    pe_q = np.asarray(pe_q, dtype=np.float32).reshape(B * H, L, D)
    pe_k = np.asarray(pe_k, dtype=np.float32).reshape(B * H, L, D)
    # pure layout packing (no compute): one DRAM tensor per stage tile keeps
    # the device-side transpose path single-dependency
    qpe = np.concatenate([q, pe_q], axis=-1)
    kpe = np.concatenate([k, pe_k], axis=-1)
    tri, ident = _host_consts()
    in_maps = []
    for c in range(NCORES):
        s = slice(c * BHPC, (c + 1) * BHPC)
        in_maps.append(
            {
                "qpe": qpe[s],
                "kpe": kpe[s],
                "v": v[s],
                "tri": tri,
                "ident": ident,
            }
        )
    return in_maps


def kernel(q, k, v, pe_q, pe_k, mask=None, **_ignored):
    """Full-input entry point: shards across 8 NeuronCores, returns full output.

    The mask input is the (fixed) causal mask of the problem; causality is
    implemented structurally in the device kernel, so it is not shipped.
    """
    _import_concourse()
    from concourse.bass_utils import run_bass_kernel_spmd

    if "nc" not in _CACHE:
        _CACHE["nc"] = _build_nc()
    nc = _CACHE["nc"]

    in_maps = _shard_inputs(q, k, v, pe_q, pe_k)
    res = run_bass_kernel_spmd(nc, in_maps, core_ids=list(range(NCORES)))
    out = np.empty((B * H, L, D), dtype=np.float32)
    for c in range(NCORES):
        out[c * BHPC : (c + 1) * BHPC] = res.results[c]["out"]
    return out.reshape(B, H, L, D)

